# revision 1
# baseline (speedup 1.0000x reference)
"""Trainium2 Bass kernel for the Euler integrator with low-rank Christoffel force.

Reference semantics (per step, fp32):
    uv  = v @ U.T                      # [B,H]
    c   = (uv*uv) @ W.T                # [B,D]
    x  += dt*v   (uses OLD v)
    v  += dt*(force - c)
    x   = mod(x + pi, 2*pi) - pi

Strategy: data-parallel over 8 NeuronCores (batch 4096 -> 512 rows/core).
All per-core tensors live transposed on chip ([feature-dim on partitions,
batch free]) so both matmuls feed the 128x128 PE array directly:
    uv[h,b] accumulates over d (2 K-tiles), stationary = U.T slice
    c[d,b]  accumulates over h (8 K-tiles), stationary = (-dt*W).T slice
Position is stored biased by +pi (cx_stored = x + pi) and accumulated
unwrapped; since |x0 + pi| < ~8.6 and |sum dt*v| < ~1.7 the value stays
inside (-2pi, 4pi), where one final comparison-mask range reduction into
[0, 2pi) reproduces the reference's per-step mod exactly (hardware has
no mod ALU op).

Matmul operands are float32r (fp32 accumulate, operands rounded to
~tf32 by the PE) which streams 1 row/cycle vs fp32's 4. Velocity keeps
a full-fp32 state tensor plus a rounded f32r copy for the matmul, so
state error does not compound at tf32 precision.
"""

import contextlib

import numpy as np

import concourse.bacc as bacc
import concourse.mybir as mybir
import concourse.tile as tile
from concourse.bass_utils import run_bass_kernel_spmd

F32 = mybir.dt.float32
F32R = mybir.dt.float32r
ALU = mybir.AluOpType
ACTF = mybir.ActivationFunctionType

N_CORES = 8
B = 4096
D = 256
H = 1024
P = 128
BS = B // N_CORES           # 512 batch rows per core
ND = D // P                 # 2 d partition-tiles
NH = H // P                 # 8 h partition-tiles

DT = np.float32(0.01 * 1.0)  # DT * DT_SCALE from the reference
PI = float(np.pi)
TWO_PI = float(2.0 * np.pi)

# matmul operand dtype: F32R (fast, ~tf32 operands) or F32 (exact, 4x slower)
MM_DT = F32R

_PROGRAM_CACHE: dict = {}


def _build(steps: int, loop_reps: int | None = None, variant: str = "full",
           uv_bufs: int = 6, dma_in_loop: bool = False, sq_dve: int = 0,
           sq_cols_dve: int = 0, b_split: bool = False, psc_bufs: int = 2,
           a_grp: int = 0):
    # loop_reps: benchmarking only — wraps the step body in a hardware For_i
    # loop so device time scales well above wall-clock noise.
    # variant: "full"/"dve" (complete kernel, all elementwise on DVE — GpSimd
    # measured ~5us/op, 25x slower than DVE, so it gets nothing) |
    # "gp" (masks+vt on GpSimd; kept for comparison) |
    # "mm_sq" (matmuls+squares only) | "mm_only" (matmuls only)
    use_gp = variant == "gp"
    do_sq = variant in ("full", "dve", "gp", "mm_sq")
    do_xv = variant in ("full", "dve", "gp")
    nc = bacc.Bacc(None, target_bir_lowering=False)

    x_d = nc.dram_tensor("xpi", [D, BS], F32, kind="ExternalInput")
    v_d = nc.dram_tensor("v", [D, BS], MM_DT, kind="ExternalInput")
    f_d = nc.dram_tensor("dtf", [D, BS], F32, kind="ExternalInput")
    u_d = nc.dram_tensor("ut", [D, H], MM_DT, kind="ExternalInput")
    w_d = nc.dram_tensor("wt", [H, D], MM_DT, kind="ExternalInput")
    xo_d = nc.dram_tensor("xo", [D, BS], F32, kind="ExternalOutput")
    vo_d = nc.dram_tensor("vo", [D, BS], F32, kind="ExternalOutput")

    with tile.TileContext(nc) as tc:
        with (
            tc.tile_pool(name="state", bufs=1) as state,
            tc.tile_pool(name="sq", bufs=16) as sqp,
            tc.tile_pool(name="tmp", bufs=4) as tmp,
            tc.tile_pool(name="psuv", bufs=uv_bufs, space="PSUM") as ps_uv,
            tc.tile_pool(name="psc", bufs=psc_bufs, space="PSUM") as ps_c,
        ):
            ut_s = [state.tile([P, H], MM_DT, name=f"ut{i}") for i in range(ND)]
            wt_s = [state.tile([P, D], MM_DT, name=f"wt{j}") for j in range(NH)]
            cx_s = [state.tile([P, BS], F32, name=f"cx{i}") for i in range(ND)]
            # full-precision velocity state + rounded matmul operand copy
            v_s = [state.tile([P, BS], F32, name=f"v{i}") for i in range(ND)]
            vr_s = [state.tile([P, BS], MM_DT, name=f"vr{i}") for i in range(ND)]
            dtf_s = [state.tile([P, BS], F32, name=f"f{i}") for i in range(ND)]

            # Input DMAs: ordered first-needed-first (v, then U chunks, then
            # W, then x/force) and round-robined across the three DMA-capable
            # queues (SP/Act HWDGE + gpsimd SWDGE) for aggregate bandwidth, so
            # the first phase-A matmuls start early and the rest streams in
            # behind compute (single-queue serial cost measured ~23us).
            def emit_input_dmas():
                xfers = []
                for i in range(ND):
                    xfers.append((vr_s[i][:], v_d[i * P:(i + 1) * P, :]))
                for j in range(NH):
                    for i in range(ND):
                        xfers.append((
                            ut_s[i][:, j * P:(j + 1) * P],
                            u_d[i * P:(i + 1) * P, j * P:(j + 1) * P],
                        ))
                for jw in range(NH):
                    xfers.append((wt_s[jw][:], w_d[jw * P:(jw + 1) * P, :]))
                for i in range(ND):
                    xfers.append((cx_s[i][:], x_d[i * P:(i + 1) * P, :]))
                    xfers.append((dtf_s[i][:], f_d[i * P:(i + 1) * P, :]))
                queues = [nc.sync, nc.gpsimd, nc.scalar]
                for k, (dst, src) in enumerate(xfers):
                    queues[k % len(queues)].dma_start(dst, src)
                for i in range(ND):
                    nc.vector.tensor_copy(v_s[i][:], vr_s[i][:].bitcast(F32))

            if not dma_in_loop:
                emit_input_dmas()

            dummy_sq = None
            if not do_sq:
                dummy_sq = [state.tile([P, BS], MM_DT, name=f"dsq{j}")
                            for j in range(NH)]
                for j in range(NH):
                    nc.sync.dma_start(dummy_sq[j][:], v_d[0:P, :])

            def emit_step():
                # ---- phase A: uv[h,b] accumulated over d, then squared.
                # Groups of a_grp h-tiles; within a group all k0 matmuls
                # issue before the k1s so the PE doesn't wait on the
                # second just-updated v d-tile at the step boundary.
                # a_grp=2 measured 3us better than 4: banks hand off to the
                # ACT squares sooner, easing uv-pool pressure, while the
                # same-bank k0->k1 spacing of 2 is still penalty-free.
                sq = []
                if a_grp == 0:
                    # hybrid: one leading pair covers the step-boundary vr1
                    # latency, then singles release banks to ACT fastest
                    groups = [[0, 1]] + [[j] for j in range(2, NH)]
                else:
                    groups = [list(range(g * a_grp, (g + 1) * a_grp))
                              for g in range(NH // a_grp)]
                for hts in groups:
                    pss = {}
                    for ht in hts:
                        ps = ps_uv.tile([P, BS], F32, tag="uv", name="uv")
                        pss[ht] = ps
                        nc.tensor.matmul(
                            ps[:], ut_s[0][:, ht * P:(ht + 1) * P],
                            vr_s[0][:], start=True, stop=False,
                        )
                    for ht in hts:
                        nc.tensor.matmul(
                            pss[ht][:], ut_s[1][:, ht * P:(ht + 1) * P],
                            vr_s[1][:], start=False, stop=True,
                        )
                        if do_sq and sq_cols_dve > 0:
                            # column-split square: ACT takes the leading
                            # columns, DVE a short copy+mul sliver, so ACT
                            # (~1.7ns/col, co-saturated with PE when it owns
                            # all 4096 cols/step) drops below the PE budget.
                            cs = BS - sq_cols_dve
                            sq_t = sqp.tile([P, BS], MM_DT, tag="sq", name="sq")
                            nc.scalar.activation(
                                sq_t[:, 0:cs], pss[ht][:, 0:cs], ACTF.Square)
                            uvt = tmp.tile([P, sq_cols_dve], F32, tag="uvt",
                                           name="uvt")
                            nc.vector.tensor_copy(uvt[:], pss[ht][:, cs:BS])
                            nc.vector.tensor_tensor(
                                out=sq_t[:, cs:BS], in0=uvt[:], in1=uvt[:],
                                op=ALU.mult,
                            )
                            sq.append(sq_t)
                        elif do_sq:
                            sq_t = sqp.tile([P, BS], MM_DT, tag="sq", name="sq")
                            if ht % 2 < sq_dve:
                                # DVE path: PSUM->SBUF copy then SBUF multiply
                                # (DVE can't read PSUM twice; ACT Square's
                                # table-based op is ~2x a DVE op and exposes
                                # ~0.7us/step when all 8 squares sit on ACT)
                                uvt = tmp.tile([P, BS], F32, tag="uvt", name="uvt")
                                nc.vector.tensor_copy(uvt[:], pss[ht][:])
                                nc.vector.tensor_tensor(
                                    out=sq_t[:], in0=uvt[:], in1=uvt[:],
                                    op=ALU.mult,
                                )
                            else:
                                nc.scalar.activation(
                                    sq_t[:], pss[ht][:], ACTF.Square)
                            sq.append(sq_t)
                        else:
                            sq.append(dummy_sq[ht])

                # ---- x-path (uses OLD v): cx += dt*v. The torus wrap is
                # deferred to one final range reduction after all steps:
                # |x0 + pi| < ~8.6 and |sum dt*v| < ~1.7, so the unwrapped
                # position stays inside (-2pi, 4pi) where a single +-2pi
                # correction equals the reference's per-step mod.
                mask_eng = nc.gpsimd if use_gp else nc.vector
                vt_s = []
                for i in range(ND):
                    if not do_xv:
                        continue
                    nc.vector.scalar_tensor_tensor(
                        out=cx_s[i][:], in0=v_s[i][:], scalar=float(DT),
                        in1=cx_s[i][:], op0=ALU.mult, op1=ALU.add,
                    )
                    # v-path part 1 (uses OLD v): vt = v + dt*force
                    vt = tmp.tile([P, BS], F32, tag="vt", name="vt")
                    mask_eng.tensor_tensor(
                        out=vt[:], in0=v_s[i][:], in1=dtf_s[i][:], op=ALU.add,
                    )
                    vt_s.append(vt)

                # ---- phase B: psc[d,b] = -dt*c over 8 h-tiles; v = vt + psc.
                # Sequential d-chains (d0's 8-matmul accumulation fully
                # before d1's) so vr0 is ready mid-phase; deep accumulation
                # chains cost ~272ns/MM vs 233 (depth-dependent: 1/2/4/8 =
                # 233/236/249/272 measured) but splitting them needs PSUM
                # banks the uv pipeline can't spare (8-bank budget).
                for i in range(ND):
                    if b_split:
                        # two 4-deep chains per d-tile (249ns/MM vs an
                        # 8-chain's 272) at the cost of an extra psc bank
                        # and one extra DVE add per output
                        psca = ps_c.tile([P, BS], F32, tag="c", name="c")
                        pscb = ps_c.tile([P, BS], F32, tag="c", name="c")
                        for j in range(NH // 2):
                            nc.tensor.matmul(
                                psca[:], wt_s[j][:, i * P:(i + 1) * P], sq[j][:],
                                start=(j == 0), stop=(j == NH // 2 - 1),
                            )
                        for j in range(NH // 2, NH):
                            nc.tensor.matmul(
                                pscb[:], wt_s[j][:, i * P:(i + 1) * P], sq[j][:],
                                start=(j == NH // 2), stop=(j == NH - 1),
                            )
                        if do_xv:
                            t1 = tmp.tile([P, BS], F32, tag="t1", name="t1")
                            nc.vector.tensor_tensor(
                                out=t1[:], in0=vt_s[i][:], in1=psca[:],
                                op=ALU.add,
                            )
                            nc.vector.tensor_tensor(
                                out=vr_s[i][:], in0=t1[:], in1=pscb[:],
                                op=ALU.add,
                            )
                            nc.vector.tensor_tensor(
                                out=v_s[i][:], in0=t1[:], in1=pscb[:],
                                op=ALU.add,
                            )
                        continue
                    psc = ps_c.tile([P, BS], F32, tag="c", name="c")
                    for j in range(NH):
                        nc.tensor.matmul(
                            psc[:], wt_s[j][:, i * P:(i + 1) * P], sq[j][:],
                            start=(j == 0), stop=(j == NH - 1),
                        )
                    if do_xv:
                        # rounded copy first so next step's phase A starts
                        # ASAP, then the full-precision fp32 state update
                        nc.vector.tensor_tensor(
                            out=vr_s[i][:], in0=vt_s[i][:], in1=psc[:], op=ALU.add,
                        )
                        nc.vector.tensor_tensor(
                            out=v_s[i][:], in0=vt_s[i][:], in1=psc[:], op=ALU.add,
                        )

            loop_cm = (
                tc.For_i(
                    0, loop_reps, 1,
                    hint_engines=(mybir.EngineType.PE, mybir.EngineType.DVE,
                                  mybir.EngineType.Activation),
                )
                if loop_reps is not None
                else contextlib.nullcontext()
            )
            with loop_cm:
                if dma_in_loop:
                    emit_input_dmas()
                for _s in range(steps):
                    emit_step()

            # final torus wrap into [0, 2pi): cx -= 2pi*(cx>=2pi) - 2pi*(cx<0)
            if do_xv:
                for i in range(ND):
                    g = tmp.tile([P, BS], F32, tag="g", name="g")
                    nc.vector.tensor_scalar(
                        out=g[:], in0=cx_s[i][:], scalar1=TWO_PI, scalar2=None,
                        op0=ALU.is_ge,
                    )
                    lo = tmp.tile([P, BS], F32, tag="l", name="l")
                    nc.vector.tensor_scalar(
                        out=lo[:], in0=cx_s[i][:], scalar1=0.0, scalar2=None,
                        op0=ALU.is_lt,
                    )
                    nc.vector.scalar_tensor_tensor(
                        out=cx_s[i][:], in0=g[:], scalar=-TWO_PI, in1=cx_s[i][:],
                        op0=ALU.mult, op1=ALU.add,
                    )
                    nc.vector.scalar_tensor_tensor(
                        out=cx_s[i][:], in0=lo[:], scalar=TWO_PI, in1=cx_s[i][:],
                        op0=ALU.mult, op1=ALU.add,
                    )

            out_queues = [nc.sync, nc.gpsimd, nc.scalar]
            for i in range(ND):
                out_queues[(2 * i) % 3].dma_start(xo_d[i * P:(i + 1) * P, :], cx_s[i][:])
                out_queues[(2 * i + 1) % 3].dma_start(vo_d[i * P:(i + 1) * P, :], v_s[i][:])

    nc.compile()
    return nc


def _get_program(steps: int, loop_reps: int | None = None, variant: str = "full",
                 **kw):
    key = (steps, loop_reps, variant, tuple(sorted(kw.items())))
    if key not in _PROGRAM_CACHE:
        _PROGRAM_CACHE[key] = _build(steps, loop_reps, variant, **kw)
    return _PROGRAM_CACHE[key]


def _run(x, v, force, U, W, steps, trace=False):
    x = np.ascontiguousarray(np.asarray(x, dtype=np.float32))
    v = np.ascontiguousarray(np.asarray(v, dtype=np.float32))
    force = np.ascontiguousarray(np.asarray(force, dtype=np.float32))
    U = np.ascontiguousarray(np.asarray(U, dtype=np.float32))
    W = np.ascontiguousarray(np.asarray(W, dtype=np.float32))
    steps = int(np.asarray(steps).item()) if not isinstance(steps, int) else steps

    if steps == 0:
        # lax.scan with length 0 returns the carry untouched (no wrap)
        return (x.copy(), v.copy()), None

    nc = _get_program(steps)

    ut = np.ascontiguousarray(U.T)                       # [D,H]
    wt = np.ascontiguousarray((-DT * W).T)               # [H,D]
    xpi = np.ascontiguousarray((x + np.float32(PI)).T)   # [D,B]
    vt = np.ascontiguousarray(v.T)                       # [D,B]
    dtf = np.ascontiguousarray((DT * force).T)           # [D,B]

    in_maps = []
    for c in range(N_CORES):
        sl = slice(c * BS, (c + 1) * BS)
        in_maps.append({
            "xpi": np.ascontiguousarray(xpi[:, sl]),
            "v": np.ascontiguousarray(vt[:, sl]),
            "dtf": np.ascontiguousarray(dtf[:, sl]),
            "ut": ut,
            "wt": wt,
        })

    try:
        res = run_bass_kernel_spmd(nc, in_maps, list(range(N_CORES)), trace=trace)
    except ModuleNotFoundError:
        # BASS_TRACE set in an env without the axon NTFF hook — retry untraced
        import os

        os.environ["BASS_NEVER_TRACE"] = "1"
        try:
            res = run_bass_kernel_spmd(nc, in_maps, list(range(N_CORES)))
        finally:
            os.environ.pop("BASS_NEVER_TRACE", None)

    xo = np.concatenate([res.results[c]["xo"].T for c in range(N_CORES)], axis=0)
    vo = np.concatenate([res.results[c]["vo"].T for c in range(N_CORES)], axis=0)
    xo = (xo - np.float32(PI)).astype(np.float32)
    return (xo, vo), res


def kernel(x, v, force, U, W, steps):
    (xo, vo), _ = _run(x, v, force, U, W, steps)
    return xo, vo



# revision 2
# speedup vs baseline: 1.0025x; 1.0025x over previous
"""Trainium2 Bass kernel v2: fp8 DoubleRow Euler integrator.

Reference semantics (per step, fp32):
    uv  = v @ U.T                      # [B,H]
    c   = (uv*uv) @ W.T                # [B,C]
    x  += dt*v   (uses OLD v)
    v  += dt*(force - c)
    x   = mod(x + pi, 2*pi) - pi

Data-parallel over 8 cores (512 batch rows each). Per-core layout is
"doubled": every [256,512] (feature, batch) slab lives on chip as
[128, 2, 512] (partition, d-tile, batch).

Design (vs the f32r baseline, 135931 ns):
  * fp8e4 DoubleRow matmuls: one instruction contracts K=256 at
    157 TF/s -- phase A is 8 matmuls/step, phase B is 2 chains of 4.
    Measured DR matmul cost ~210-250 ns vs f32r's ~270-300, and half
    the instruction count of the f32r kernel.
  * Weight quantization error is the accuracy bottleneck (systematic
    across steps). Each of U/W ships as FOUR fp8 variants cycled per
    step: antithetic pairs (b = fp8(2x - a), so consecutive steps'
    rounding errors cancel to first order) on two shifted grids
    (x*1.0905 rounds on a different lattice, decorrelating the pair
    residuals). The grid gain is exactly compensated by per-step
    immediates (ACT square scale, v-update scalar, force-preload
    identity value). Simulated rel err 8.6e-3 (gate 2e-2).
  * x never touches DVE in the loop: a dt-scaled identity f32r matmul
    accumulates sum(dt*v_old) into a persistent PSUM bank pair; x0 is
    added once after the loop.
  * g*8*force is preloaded into the c-accumulator via an identity
    matmul heading each phase-B chain, so the v update is one
    scalar_tensor_tensor (v += psc*dt/(8g)) plus a second writing the
    fp8 operand copy for the next step's phase A.
  * squares run fused per h-pair on ACT ([128,2,512] PSUM -> fp8 SBUF
    in exactly the DoubleRow moving layout, ~0.93 ns/col measured),
    with an optional trailing-column DVE offload knob.
"""

import contextlib

import numpy as np
import ml_dtypes

import concourse.bacc as bacc
import concourse.mybir as mybir
import concourse.tile as tile
from concourse.bass_utils import run_bass_kernel_spmd

F32 = mybir.dt.float32
F32R = mybir.dt.float32r
FP8 = mybir.dt.float8e4
NP_FP8 = ml_dtypes.float8_e4m3
ALU = mybir.AluOpType
ACTF = mybir.ActivationFunctionType
DR = mybir.MatmulPerfMode.DoubleRow

N_CORES = 8
B = 4096
D = 256
H = 1024
P = 128
BS = B // N_CORES           # 512 batch rows per core
NH = H // P                 # 8 h partition-tiles -> 4 DR pairs
NV = 4                      # quantization variants of U/W

DT = np.float32(0.01 * 1.0)
PI = float(np.pi)
TWO_PI = float(2.0 * np.pi)

SU = 8.0                    # U pre-scale before fp8 quantization
SW = 8.0                    # W pre-scale (also the force-preload gain)
GAM = 1.0905                # second-quantization-grid gain
GS = [1.0, 1.0, GAM, GAM]   # per-variant grid gain

_PROGRAM_CACHE: dict = {}


def _build(steps: int, loop_reps: int | None = None, variant: str = "full",
           sq_dve_cols: int = 0, uv_bufs: int = 2, sq_bufs: int = 3,
           dve_pairs: int = 0):
    # variant: "full" | "mm_only" (matmuls with precomputed dummy sq; no
    # squares, no state updates) | "mm_sq" (matmuls + squares, no state)
    # sq_dve_cols: trailing batch columns of each half that DVE squares
    # (copy+mult) instead of ACT.  dve_pairs: number of whole h-pairs
    # (from the back) whose squares run fully on DVE.
    do_sq = variant in ("full", "mm_sq")
    do_xv = variant == "full"
    nc = bacc.Bacc(None, target_bir_lowering=False)

    x_d = nc.dram_tensor("xpi", [P, 2, BS], F32, kind="ExternalInput")
    # v state is tagged F32R (bit-identical to f32) so the x-accumulation
    # identity matmul can consume it; DVE ops bitcast it back to F32.
    v_d = nc.dram_tensor("v", [P, 2, BS], F32R, kind="ExternalInput")
    vr_d = nc.dram_tensor("vr", [P, 2, BS], FP8, kind="ExternalInput")
    f_d = nc.dram_tensor("f", [P, 2, BS], F32R, kind="ExternalInput")
    ut_d = [nc.dram_tensor(f"ut{i}", [P, 2, H], FP8, kind="ExternalInput")
            for i in range(NV)]
    wt_d = [nc.dram_tensor(f"wt{i}", [P, NH, D], FP8, kind="ExternalInput")
            for i in range(NV)]
    dti_d = nc.dram_tensor("dti", [P, P], F32R, kind="ExternalInput")
    # force-preload identities: value 8 (variants 0,1) and 8*GAM (2,3)
    ei8_d = [nc.dram_tensor(f"ei8{g}", [P, P], F32R, kind="ExternalInput")
             for g in range(2)]
    xo_d = nc.dram_tensor("xo", [P, 2, BS], F32, kind="ExternalOutput")
    vo_d = nc.dram_tensor("vo", [P, 2, BS], F32, kind="ExternalOutput")

    with tile.TileContext(nc) as tc:
        with (
            tc.tile_pool(name="state", bufs=1) as state,
            tc.tile_pool(name="sq", bufs=sq_bufs) as sqp,
            tc.tile_pool(name="tmp", bufs=4) as tmp,
            tc.tile_pool(name="psuv", bufs=uv_bufs, space="PSUM") as ps_uv,
            tc.tile_pool(name="psc", bufs=1, space="PSUM") as ps_c,
            tc.tile_pool(name="psx", bufs=1, space="PSUM") as ps_x,
        ):
            ut_s = [state.tile([P, 2, H], FP8, name=f"ut{i}")
                    for i in range(NV)]
            wt_s = [state.tile([P, NH, D], FP8, name=f"wt{i}")
                    for i in range(NV)]
            v_s = state.tile([P, 2, BS], F32R, name="v")
            vr_s = state.tile([P, 2, BS], FP8, name="vr")
            f_s = state.tile([P, 2, BS], F32R, name="f")
            x_s = state.tile([P, 2, BS], F32, name="xpi")
            dti_s = state.tile([P, P], F32R, name="dti")
            ei8_s = [state.tile([P, P], F32R, name=f"ei8{g}") for g in range(2)]
            xacc = ps_x.tile([P, 2, BS], F32, name="xacc")

            # Inputs ordered first-needed-first, round-robined across the
            # three DMA-capable queues for aggregate bandwidth.
            xfers = [(vr_s, vr_d), (ut_s[0], ut_d[0]), (v_s, v_d),
                     (dti_s, dti_d), (ei8_s[0], ei8_d[0]),
                     (ei8_s[1], ei8_d[1]), (f_s, f_d),
                     (wt_s[0], wt_d[0])]
            for i in range(1, NV):
                xfers += [(ut_s[i], ut_d[i]), (wt_s[i], wt_d[i])]
            xfers.append((x_s, x_d))
            queues = [nc.sync, nc.gpsimd, nc.scalar]
            for k, (dst, src) in enumerate(xfers):
                queues[k % len(queues)].dma_start(dst[:], src[:])

            dummy_sq = None
            if not do_sq:
                dummy_sq = [state.tile([P, 2, BS], FP8, name=f"dsq{j}")
                            for j in range(NH // 2)]
                for j in range(NH // 2):
                    nc.sync.dma_start(dummy_sq[j][:], vr_d[:])

            def emit_pair_A(pp, ut):
                # uv for h-tiles (2pp, 2pp+1): one DoubleRow matmul per
                # h-tile contracts both d-tiles (K=256).
                uvp = ps_uv.tile([P, 2, BS], F32, tag="uv", name="uv")
                for m in range(2):
                    ht = 2 * pp + m
                    hs = slice(ht * P, (ht + 1) * P)
                    nc.tensor.matmul(
                        uvp[:, m, :], ut[:, :, hs], vr_s[:],
                        start=True, stop=True, perf_mode=DR,
                    )
                return uvp

            def emit_sq(pp, uvp, on_dve, ga):
                # square into the DR moving layout; optional column split
                # between ACT (leading) and DVE (trailing, copy+mult).
                sq_t = sqp.tile([P, 2, BS], FP8, tag="sq", name="sq")
                cs = 0 if on_dve else BS - sq_dve_cols
                if cs > 0:
                    nc.scalar.activation(
                        sq_t[:, :, 0:cs], uvp[:, :, 0:cs], ACTF.Square,
                        scale=float(1.0 / (SU * ga)),
                    )
                if cs < BS:
                    uvt = tmp.tile([P, 2, BS - cs], F32, tag="uvt", name="uvt")
                    nc.vector.tensor_scalar(
                        out=uvt[:], in0=uvp[:, :, cs:BS],
                        scalar1=float(1.0 / (SU * ga)), scalar2=None,
                        op0=ALU.mult,
                    )
                    nc.vector.tensor_tensor(
                        out=sq_t[:, :, cs:BS], in0=uvt[:], in1=uvt[:],
                        op=ALU.mult,
                    )
                return sq_t

            def emit_step(s):
                sv = s % NV
                ga = GS[sv]
                ut, wt = ut_s[sv], wt_s[sv]
                ei8 = ei8_s[0] if sv < 2 else ei8_s[1]
                vs_im = float(DT / (SW * ga))

                uvs = [emit_pair_A(0, ut), emit_pair_A(1, ut), None, None]
                psc = ps_c.tile([P, 2, BS], F32, tag="c", name="c")
                for k in range(2):
                    # force preload: psc = 8*g*force (f32r identity matmul)
                    nc.tensor.matmul(
                        psc[:, k, :], ei8[:], f_s[:, k, :],
                        start=True, stop=False,
                    )
                for pp in range(NH // 2):
                    uvp = uvs[pp] if uvs[pp] is not None else None
                    if uvp is None:
                        uvp = emit_pair_A(pp, ut)
                    if do_sq:
                        sq_t = emit_sq(pp, uvp, pp >= NH // 2 - dve_pairs, ga)
                    else:
                        sq_t = dummy_sq[pp]
                    for k in range(2):
                        ds = slice(k * P, (k + 1) * P)
                        nc.tensor.matmul(
                            psc[:, k, :], wt[:, 2 * pp:2 * pp + 2, ds],
                            sq_t[:], start=False,
                            stop=(pp == NH // 2 - 1), perf_mode=DR,
                        )
                    if pp + 2 < NH // 2:
                        uvs[pp + 2] = emit_pair_A(pp + 2, ut)
                if do_xv:
                    # x-acc matmuls sit at the END of the step's PE stream:
                    # they read v_old and only gate this step's v update, so
                    # emitting them early would stall the in-order PE queue
                    # on the previous step's v write.
                    for k in range(2):
                        nc.tensor.matmul(
                            xacc[:, k, :], dti_s[:], v_s[:, k, :],
                            start=(s == 0), stop=(s == steps - 1),
                        )
                    # fp8 operand copy first so next step's phase A starts
                    # ASAP, then the fp32 state update in place.
                    nc.vector.scalar_tensor_tensor(
                        out=vr_s[:], in0=psc[:], scalar=vs_im,
                        in1=v_s[:].bitcast(F32), op0=ALU.mult, op1=ALU.add,
                    )
                    nc.vector.scalar_tensor_tensor(
                        out=v_s[:], in0=psc[:], scalar=vs_im,
                        in1=v_s[:].bitcast(F32), op0=ALU.mult, op1=ALU.add,
                    )

            loop_cm = (
                tc.For_i(
                    0, loop_reps, 1,
                    hint_engines=(mybir.EngineType.PE, mybir.EngineType.DVE,
                                  mybir.EngineType.Activation),
                )
                if loop_reps is not None
                else contextlib.nullcontext()
            )
            with loop_cm:
                for s in range(steps):
                    emit_step(s)

            # x epilogue: cx = (x0+pi) + sum(dt*v), then one range
            # reduction into [0, 2pi) replicating the per-step mod chain.
            if do_xv:
                cx = tmp.tile([P, 2, BS], F32, tag="cx", name="cx")
                nc.vector.tensor_tensor(
                    out=cx[:], in0=x_s[:], in1=xacc[:], op=ALU.add,
                )
                g = tmp.tile([P, 2, BS], F32, tag="g", name="g")
                nc.vector.tensor_scalar(
                    out=g[:], in0=cx[:], scalar1=TWO_PI, scalar2=None,
                    op0=ALU.is_ge,
                )
                lo = tmp.tile([P, 2, BS], F32, tag="l", name="l")
                nc.vector.tensor_scalar(
                    out=lo[:], in0=cx[:], scalar1=0.0, scalar2=None,
                    op0=ALU.is_lt,
                )
                nc.vector.scalar_tensor_tensor(
                    out=cx[:], in0=g[:], scalar=-TWO_PI, in1=cx[:],
                    op0=ALU.mult, op1=ALU.add,
                )
                nc.vector.scalar_tensor_tensor(
                    out=cx[:], in0=lo[:], scalar=TWO_PI, in1=cx[:],
                    op0=ALU.mult, op1=ALU.add,
                )
                nc.sync.dma_start(xo_d[:], cx[:])
                nc.gpsimd.dma_start(vo_d[:], v_s[:].bitcast(F32))
            else:
                nc.sync.dma_start(xo_d[:], x_s[:])
                nc.gpsimd.dma_start(vo_d[:], v_s[:].bitcast(F32))

    nc.compile()
    return nc


def _get_program(steps: int, loop_reps: int | None = None, variant: str = "full",
                 **kw):
    key = (steps, loop_reps, variant, tuple(sorted(kw.items())))
    if key not in _PROGRAM_CACHE:
        _PROGRAM_CACHE[key] = _build(steps, loop_reps, variant, **kw)
    return _PROGRAM_CACHE[key]


def _double(a):
    # [256, n] -> [128, 2, n]: row d = i*128+p lands at [p, i, :]
    n = a.shape[1]
    return np.ascontiguousarray(a.reshape(2, P, n).transpose(1, 0, 2))


def _undouble(a):
    # [128, 2, n] -> [256, n]
    n = a.shape[2]
    return np.ascontiguousarray(a.transpose(1, 0, 2).reshape(2 * P, n))


def _fp8(a):
    return np.ascontiguousarray(np.asarray(a, np.float32).astype(NP_FP8))


def _variants(tf):
    # 4 fp8 quantizations: antithetic pairs on two shifted grids
    a = _fp8(tf)
    b = _fp8(2.0 * tf - a.astype(np.float32))
    c = _fp8(tf * np.float32(GAM))
    dv = _fp8(2.0 * tf * np.float32(GAM) - c.astype(np.float32))
    return [a, b, c, dv]


def make_in_maps(x, v, force, U, W):
    ut3 = (SU * U.T).astype(np.float32).reshape(2, P, H).transpose(1, 0, 2)
    wt3 = (-SW * W.T).astype(np.float32).reshape(NH, P, D).transpose(1, 0, 2)
    uts = _variants(np.ascontiguousarray(ut3))
    wts = _variants(np.ascontiguousarray(wt3))
    dti = np.ascontiguousarray(DT * np.eye(P, dtype=np.float32))
    ei8s = [np.ascontiguousarray(np.float32(SW * g) * np.eye(P, dtype=np.float32))
            for g in (1.0, GAM)]

    xpi = (x + np.float32(PI)).T.astype(np.float32)   # [D,B]
    vt = v.T.astype(np.float32)
    ft = force.T.astype(np.float32)

    in_maps = []
    for core in range(N_CORES):
        sl = slice(core * BS, (core + 1) * BS)
        v2 = _double(vt[:, sl])
        m = {
            "xpi": _double(xpi[:, sl]),
            "v": v2,
            "vr": _fp8(v2),
            "f": _double(ft[:, sl]),
            "dti": dti, "ei80": ei8s[0], "ei81": ei8s[1],
        }
        for i in range(NV):
            m[f"ut{i}"] = uts[i]
            m[f"wt{i}"] = wts[i]
        in_maps.append(m)
    return in_maps


def _run(x, v, force, U, W, steps, trace=False, **build_kw):
    x = np.ascontiguousarray(np.asarray(x, dtype=np.float32))
    v = np.ascontiguousarray(np.asarray(v, dtype=np.float32))
    force = np.ascontiguousarray(np.asarray(force, dtype=np.float32))
    U = np.ascontiguousarray(np.asarray(U, dtype=np.float32))
    W = np.ascontiguousarray(np.asarray(W, dtype=np.float32))
    steps = int(np.asarray(steps).item()) if not isinstance(steps, int) else steps

    if steps == 0:
        return (x.copy(), v.copy()), None

    nc = _get_program(steps, **build_kw)
    in_maps = make_in_maps(x, v, force, U, W)

    try:
        res = run_bass_kernel_spmd(nc, in_maps, list(range(N_CORES)), trace=trace)
    except ModuleNotFoundError:
        import os

        os.environ["BASS_NEVER_TRACE"] = "1"
        try:
            res = run_bass_kernel_spmd(nc, in_maps, list(range(N_CORES)))
        finally:
            os.environ.pop("BASS_NEVER_TRACE", None)

    xo = np.concatenate(
        [_undouble(res.results[c]["xo"]).T for c in range(N_CORES)], axis=0)
    vo = np.concatenate(
        [_undouble(res.results[c]["vo"]).T for c in range(N_CORES)], axis=0)
    xo = (xo - np.float32(PI)).astype(np.float32)
    return (xo, vo), res


def kernel(x, v, force, U, W, steps):
    (xo, vo), _ = _run(x, v, force, U, W, steps)
    return xo, vo


# revision 3
# speedup vs baseline: 1.0229x; 1.0203x over previous
"""Trainium2 Bass kernel v2: fp8 DoubleRow Euler integrator.

Reference semantics (per step, fp32):
    uv  = v @ U.T                      # [B,H]
    c   = (uv*uv) @ W.T                # [B,C]
    x  += dt*v   (uses OLD v)
    v  += dt*(force - c)
    x   = mod(x + pi, 2*pi) - pi

Data-parallel over 8 cores (512 batch rows each). Per-core layout is
"doubled": every [256,512] (feature, batch) slab lives on chip as
[128, 2, 512] (partition, d-tile, batch).

Design (vs the f32r baseline, 135931 ns):
  * fp8e4 DoubleRow matmuls: one instruction contracts K=256 at
    157 TF/s -- phase A is 8 matmuls/step, phase B is 2 chains of 4.
    Measured DR matmul cost ~210-250 ns vs f32r's ~270-300, and half
    the instruction count of the f32r kernel.
  * Weight quantization error is the accuracy bottleneck (systematic
    across steps). Each of U/W ships as FOUR fp8 variants cycled per
    step: antithetic pairs (b = fp8(2x - a), so consecutive steps'
    rounding errors cancel to first order) on two shifted grids
    (x*1.0905 rounds on a different lattice, decorrelating the pair
    residuals). The grid gain is exactly compensated by per-step
    immediates (ACT square scale, v-update scalar, force-preload
    identity value). Simulated rel err 8.6e-3 (gate 2e-2).
  * x never touches DVE in the loop: a dt-scaled identity f32r matmul
    accumulates sum(dt*v_old) into a persistent PSUM bank pair; x0 is
    added once after the loop.
  * g*8*force is preloaded into the c-accumulator via an identity
    matmul heading each phase-B chain, so the v update is one
    scalar_tensor_tensor (v += psc*dt/(8g)) plus a second writing the
    fp8 operand copy for the next step's phase A.
  * squares run fused per h-pair on ACT ([128,2,512] PSUM -> fp8 SBUF
    in exactly the DoubleRow moving layout, ~0.93 ns/col measured),
    with an optional trailing-column DVE offload knob.
"""

import contextlib

import numpy as np
import ml_dtypes

import concourse.bacc as bacc
import concourse.mybir as mybir
import concourse.tile as tile
from concourse.bass_utils import run_bass_kernel_spmd

F32 = mybir.dt.float32
F32R = mybir.dt.float32r
FP8 = mybir.dt.float8e4
NP_FP8 = ml_dtypes.float8_e4m3
ALU = mybir.AluOpType
ACTF = mybir.ActivationFunctionType
DR = mybir.MatmulPerfMode.DoubleRow

N_CORES = 8
B = 4096
D = 256
H = 1024
P = 128
BS = B // N_CORES           # 512 batch rows per core
NH = H // P                 # 8 h partition-tiles -> 4 DR pairs
NV = 4                      # quantization variants of U/W

DT = np.float32(0.01 * 1.0)
PI = float(np.pi)
TWO_PI = float(2.0 * np.pi)

SU = 8.0                    # U pre-scale before fp8 quantization
SW = 8.0                    # W pre-scale (also the force-preload gain)
GAM = 1.0905                # second-quantization-grid gain
GS = [1.0, 1.0, GAM, GAM]   # per-variant grid gain

_PROGRAM_CACHE: dict = {}


def _build(steps: int, loop_reps: int | None = None, variant: str = "full",
           sq_dve_cols: int = 0, uv_bufs: int = 2, sq_bufs: int = 3,
           dve_pairs: int = 0, hsplit: bool = False):
    # variant: "full" | "mm_only" (matmuls with precomputed dummy sq; no
    # squares, no state updates) | "mm_sq" (matmuls + squares, no state)
    # sq_dve_cols: trailing batch columns of each half that DVE squares
    # (copy+mult) instead of ACT.  dve_pairs: number of whole h-pairs
    # (from the back) whose squares run fully on DVE.
    do_sq = variant in ("full", "mm_sq")
    do_xv = variant == "full"
    nc = bacc.Bacc(None, target_bir_lowering=False)

    x_d = nc.dram_tensor("xpi", [P, 2, BS], F32, kind="ExternalInput")
    # v state is tagged F32R (bit-identical to f32) so the x-accumulation
    # identity matmul can consume it; DVE ops bitcast it back to F32.
    v_d = nc.dram_tensor("v", [P, 2, BS], F32R, kind="ExternalInput")
    vr_d = nc.dram_tensor("vr", [P, 2, BS], FP8, kind="ExternalInput")
    f_d = nc.dram_tensor("f", [P, 2, BS], F32R, kind="ExternalInput")
    ut_d = [nc.dram_tensor(f"ut{i}", [P, 2, H], FP8, kind="ExternalInput")
            for i in range(NV)]
    wt_d = [nc.dram_tensor(f"wt{i}", [P, NH, D], FP8, kind="ExternalInput")
            for i in range(NV)]
    dti_d = nc.dram_tensor("dti", [P, P], F32R, kind="ExternalInput")
    # force-preload identities: value 8 (variants 0,1) and 8*GAM (2,3)
    ei8_d = [nc.dram_tensor(f"ei8{g}", [P, P], F32R, kind="ExternalInput")
             for g in range(2)]
    # v-fold identities: 1/VS = 800g, so psc carries (v/VS + 8g(f - c))
    # and the vr/v updates become single-input scaled copies.
    iv_d = [nc.dram_tensor(f"iv{g}", [P, P], F32R, kind="ExternalInput")
            for g in range(2)]
    xo_d = nc.dram_tensor("xo", [P, 2, BS], F32, kind="ExternalOutput")
    vo_d = nc.dram_tensor("vo", [P, 2, BS], F32, kind="ExternalOutput")

    with tile.TileContext(nc) as tc:
        with (
            tc.tile_pool(name="state", bufs=1) as state,
            tc.tile_pool(name="sq", bufs=sq_bufs) as sqp,
            tc.tile_pool(name="tmp", bufs=4) as tmp,
            tc.tile_pool(name="psuv", bufs=uv_bufs, space="PSUM") as ps_uv,
            tc.tile_pool(name="psc", bufs=1, space="PSUM") as ps_c,
            tc.tile_pool(name="psx", bufs=1, space="PSUM") as ps_x,
        ):
            ut_s = [state.tile([P, 2, H], FP8, name=f"ut{i}")
                    for i in range(NV)]
            wt_s = [state.tile([P, NH, D], FP8, name=f"wt{i}")
                    for i in range(NV)]
            v_s = state.tile([P, 2, BS], F32R, name="v")
            vr_s = state.tile([P, 2, BS], FP8, name="vr")
            f_s = state.tile([P, 2, BS], F32R, name="f")
            x_s = state.tile([P, 2, BS], F32, name="xpi")
            dti_s = state.tile([P, P], F32R, name="dti")
            ei8_s = [state.tile([P, P], F32R, name=f"ei8{g}") for g in range(2)]
            iv_s = [state.tile([P, P], F32R, name=f"iv{g}") for g in range(2)]
            xacc = ps_x.tile([P, 2, BS], F32, name="xacc")

            # Inputs ordered first-needed-first, round-robined across the
            # three DMA-capable queues for aggregate bandwidth.
            xfers = [(vr_s, vr_d), (ut_s[0], ut_d[0]), (v_s, v_d),
                     (dti_s, dti_d), (ei8_s[0], ei8_d[0]),
                     (ei8_s[1], ei8_d[1]), (f_s, f_d),
                     (iv_s[0], iv_d[0]), (iv_s[1], iv_d[1]),
                     (wt_s[0], wt_d[0])]
            for i in range(1, NV):
                xfers += [(ut_s[i], ut_d[i]), (wt_s[i], wt_d[i])]
            xfers.append((x_s, x_d))
            queues = [nc.sync, nc.gpsimd, nc.scalar]
            for k, (dst, src) in enumerate(xfers):
                queues[k % len(queues)].dma_start(dst[:], src[:])

            dummy_sq = None
            if not do_sq:
                dummy_sq = [state.tile([P, 2, BS], FP8, name=f"dsq{j}")
                            for j in range(NH // 2)]
                for j in range(NH // 2):
                    nc.sync.dma_start(dummy_sq[j][:], vr_d[:])

            def emit_pair_A(pp, ut, half=None):
                # uv for h-tiles (2pp, 2pp+1): one DoubleRow matmul per
                # h-tile contracts both d-tiles (K=256). With half-split,
                # each h-tile runs as two batch-half matmuls gated on the
                # corresponding vr half so phase A starts right after the
                # first half of the v update lands.
                uvp = ps_uv.tile([P, 2, BS], F32, tag="uv", name="uv")
                halves = [(0, BS)] if half is None else half
                for b0, b1 in halves:
                    for m in range(2):
                        ht = 2 * pp + m
                        hs = slice(ht * P, (ht + 1) * P)
                        nc.tensor.matmul(
                            uvp[:, m, b0:b1], ut[:, :, hs],
                            vr_s[:, :, b0:b1],
                            start=True, stop=True, perf_mode=DR,
                        )
                return uvp

            def emit_sq(pp, uvp, on_dve, ga):
                # square into the DR moving layout; optional column split
                # between ACT (leading) and DVE (trailing, copy+mult).
                sq_t = sqp.tile([P, 2, BS], FP8, tag="sq", name="sq")
                cs = 0 if on_dve else BS - sq_dve_cols
                if cs > 0 and hsplit:
                    hb = cs // 2
                    nc.scalar.activation(
                        sq_t[:, :, 0:hb], uvp[:, :, 0:hb], ACTF.Square,
                        scale=float(1.0 / (SU * ga)),
                    )
                    nc.scalar.activation(
                        sq_t[:, :, hb:cs], uvp[:, :, hb:cs], ACTF.Square,
                        scale=float(1.0 / (SU * ga)),
                    )
                elif cs > 0:
                    nc.scalar.activation(
                        sq_t[:, :, 0:cs], uvp[:, :, 0:cs], ACTF.Square,
                        scale=float(1.0 / (SU * ga)),
                    )
                if cs < BS:
                    uvt = tmp.tile([P, 2, BS - cs], F32, tag="uvt", name="uvt")
                    nc.vector.tensor_scalar(
                        out=uvt[:], in0=uvp[:, :, cs:BS],
                        scalar1=float(1.0 / (SU * ga)), scalar2=None,
                        op0=ALU.mult,
                    )
                    nc.vector.tensor_tensor(
                        out=sq_t[:, :, cs:BS], in0=uvt[:], in1=uvt[:],
                        op=ALU.mult,
                    )
                return sq_t

            def emit_step(s):
                sv = s % NV
                ga = GS[sv]
                ut, wt = ut_s[sv], wt_s[sv]
                ei8 = ei8_s[0] if sv < 2 else ei8_s[1]
                iv = iv_s[0] if sv < 2 else iv_s[1]
                vs_im = float(DT / (SW * ga))

                hv = [(0, BS // 2), (BS // 2, BS)] if hsplit else None
                uvs = [emit_pair_A(0, ut, hv), emit_pair_A(1, ut, hv),
                       None, None]
                psc = ps_c.tile([P, 2, BS], F32, tag="c", name="c")
                for k in range(2):
                    # preload psc = 8g*force + (800g)*v_old, so the state
                    # updates below are single-input scaled copies of psc
                    nc.tensor.matmul(
                        psc[:, k, :], ei8[:], f_s[:, k, :],
                        start=True, stop=False,
                    )
                    if do_xv:
                        nc.tensor.matmul(
                            psc[:, k, :], iv[:], v_s[:, k, :],
                            start=False, stop=False,
                        )
                for pp in range(NH // 2):
                    uvp = uvs[pp] if uvs[pp] is not None else None
                    if uvp is None:
                        uvp = emit_pair_A(pp, ut)
                    if do_sq:
                        sq_t = emit_sq(pp, uvp, pp >= NH // 2 - dve_pairs, ga)
                    else:
                        sq_t = dummy_sq[pp]
                    for k in range(2):
                        ds = slice(k * P, (k + 1) * P)
                        nc.tensor.matmul(
                            psc[:, k, :], wt[:, 2 * pp:2 * pp + 2, ds],
                            sq_t[:], start=False,
                            stop=(pp == NH // 2 - 1), perf_mode=DR,
                        )
                    if pp + 2 < NH // 2:
                        uvs[pp + 2] = emit_pair_A(pp + 2, ut, hv)
                if do_xv:
                    # vr = fp8(VS*psc) is a pure scaled copy now: it runs on
                    # ACT right behind the last square (same-engine, no
                    # cross-engine hop on the critical chain).
                    nc.scalar.activation(
                        vr_s[:], psc[:], ACTF.Copy, scale=vs_im,
                    )
                    # x-acc matmuls at the END of the PE stream: they read
                    # v_old and only gate this step's v write.
                    for k in range(2):
                        nc.tensor.matmul(
                            xacc[:, k, :], dti_s[:], v_s[:, k, :],
                            start=(s == 0), stop=(s == steps - 1),
                        )
                    # f32 state update off-chain on DVE
                    nc.vector.tensor_scalar(
                        out=v_s[:], in0=psc[:], scalar1=vs_im, scalar2=None,
                        op0=ALU.mult,
                    )

            loop_cm = (
                tc.For_i(
                    0, loop_reps, 1,
                    hint_engines=(mybir.EngineType.PE, mybir.EngineType.DVE,
                                  mybir.EngineType.Activation),
                )
                if loop_reps is not None
                else contextlib.nullcontext()
            )
            with loop_cm:
                for s in range(steps):
                    emit_step(s)

            # x epilogue: cx = (x0+pi) + sum(dt*v), then one range
            # reduction into [0, 2pi) replicating the per-step mod chain.
            if do_xv:
                cx = tmp.tile([P, 2, BS], F32, tag="cx", name="cx")
                nc.vector.tensor_tensor(
                    out=cx[:], in0=x_s[:], in1=xacc[:], op=ALU.add,
                )
                g = tmp.tile([P, 2, BS], F32, tag="g", name="g")
                nc.vector.tensor_scalar(
                    out=g[:], in0=cx[:], scalar1=TWO_PI, scalar2=None,
                    op0=ALU.is_ge,
                )
                lo = tmp.tile([P, 2, BS], F32, tag="l", name="l")
                nc.vector.tensor_scalar(
                    out=lo[:], in0=cx[:], scalar1=0.0, scalar2=None,
                    op0=ALU.is_lt,
                )
                nc.vector.scalar_tensor_tensor(
                    out=cx[:], in0=g[:], scalar=-TWO_PI, in1=cx[:],
                    op0=ALU.mult, op1=ALU.add,
                )
                nc.vector.scalar_tensor_tensor(
                    out=cx[:], in0=lo[:], scalar=TWO_PI, in1=cx[:],
                    op0=ALU.mult, op1=ALU.add,
                )
                nc.sync.dma_start(xo_d[:], cx[:])
                nc.gpsimd.dma_start(vo_d[:], v_s[:].bitcast(F32))
            else:
                nc.sync.dma_start(xo_d[:], x_s[:])
                nc.gpsimd.dma_start(vo_d[:], v_s[:].bitcast(F32))

    nc.compile()
    return nc


def _get_program(steps: int, loop_reps: int | None = None, variant: str = "full",
                 **kw):
    key = (steps, loop_reps, variant, tuple(sorted(kw.items())))
    if key not in _PROGRAM_CACHE:
        _PROGRAM_CACHE[key] = _build(steps, loop_reps, variant, **kw)
    return _PROGRAM_CACHE[key]


def _double(a):
    # [256, n] -> [128, 2, n]: row d = i*128+p lands at [p, i, :]
    n = a.shape[1]
    return np.ascontiguousarray(a.reshape(2, P, n).transpose(1, 0, 2))


def _undouble(a):
    # [128, 2, n] -> [256, n]
    n = a.shape[2]
    return np.ascontiguousarray(a.transpose(1, 0, 2).reshape(2 * P, n))


def _fp8(a):
    return np.ascontiguousarray(np.asarray(a, np.float32).astype(NP_FP8))


def _variants(tf):
    # 4 fp8 quantizations: antithetic pairs on two shifted grids
    a = _fp8(tf)
    b = _fp8(2.0 * tf - a.astype(np.float32))
    c = _fp8(tf * np.float32(GAM))
    dv = _fp8(2.0 * tf * np.float32(GAM) - c.astype(np.float32))
    return [a, b, c, dv]


def make_in_maps(x, v, force, U, W):
    ut3 = (SU * U.T).astype(np.float32).reshape(2, P, H).transpose(1, 0, 2)
    wt3 = (-SW * W.T).astype(np.float32).reshape(NH, P, D).transpose(1, 0, 2)
    uts = _variants(np.ascontiguousarray(ut3))
    wts = _variants(np.ascontiguousarray(wt3))
    dti = np.ascontiguousarray(DT * np.eye(P, dtype=np.float32))
    ei8s = [np.ascontiguousarray(np.float32(SW * g) * np.eye(P, dtype=np.float32))
            for g in (1.0, GAM)]
    ivs = [np.ascontiguousarray(np.float32(SW * g / DT) * np.eye(P, dtype=np.float32))
           for g in (1.0, GAM)]

    xpi = (x + np.float32(PI)).T.astype(np.float32)   # [D,B]
    vt = v.T.astype(np.float32)
    ft = force.T.astype(np.float32)

    in_maps = []
    for core in range(N_CORES):
        sl = slice(core * BS, (core + 1) * BS)
        v2 = _double(vt[:, sl])
        m = {
            "xpi": _double(xpi[:, sl]),
            "v": v2,
            "vr": _fp8(v2),
            "f": _double(ft[:, sl]),
            "dti": dti, "ei80": ei8s[0], "ei81": ei8s[1],
            "iv0": ivs[0], "iv1": ivs[1],
        }
        for i in range(NV):
            m[f"ut{i}"] = uts[i]
            m[f"wt{i}"] = wts[i]
        in_maps.append(m)
    return in_maps


def _run(x, v, force, U, W, steps, trace=False, **build_kw):
    x = np.ascontiguousarray(np.asarray(x, dtype=np.float32))
    v = np.ascontiguousarray(np.asarray(v, dtype=np.float32))
    force = np.ascontiguousarray(np.asarray(force, dtype=np.float32))
    U = np.ascontiguousarray(np.asarray(U, dtype=np.float32))
    W = np.ascontiguousarray(np.asarray(W, dtype=np.float32))
    steps = int(np.asarray(steps).item()) if not isinstance(steps, int) else steps

    if steps == 0:
        return (x.copy(), v.copy()), None

    nc = _get_program(steps, **build_kw)
    in_maps = make_in_maps(x, v, force, U, W)

    try:
        res = run_bass_kernel_spmd(nc, in_maps, list(range(N_CORES)), trace=trace)
    except ModuleNotFoundError:
        import os

        os.environ["BASS_NEVER_TRACE"] = "1"
        try:
            res = run_bass_kernel_spmd(nc, in_maps, list(range(N_CORES)))
        finally:
            os.environ.pop("BASS_NEVER_TRACE", None)

    xo = np.concatenate(
        [_undouble(res.results[c]["xo"]).T for c in range(N_CORES)], axis=0)
    vo = np.concatenate(
        [_undouble(res.results[c]["vo"]).T for c in range(N_CORES)], axis=0)
    xo = (xo - np.float32(PI)).astype(np.float32)
    return (xo, vo), res


def kernel(x, v, force, U, W, steps):
    (xo, vo), _ = _run(x, v, force, U, W, steps)
    return xo, vo
